# revision 20
# baseline (speedup 1.0000x reference)
"""3-layer GCN on 8 Trainium2 NeuronCores (Bass/Tile).

Math (per layer, identical to PyG GCNConv with self-loops):
    x_{l+1} = A_hat @ (x_l @ W_l) + b_l,   A_hat = D^-1/2 (A+I) D^-1/2

Key restructurings vs the straightforward form:
  * Aggregate first, GEMM second (associativity): each core runs the
    128x128 GEMM only on its own 1/8 of the nodes.
  * Norm folding: the gather table stores t_l = D^-1/2 x_l, so the
    aggregation matrix is the 0/1 adjacency (S tiles need no norm
    column) and the D^-1/2 on the output side becomes a per-partition
    scale at the PSUM->SBUF copy.  The bias rides a rank-1 matmul
    invd (x) b so the same scale produces either t_{l+1} (layers 0,1)
    or the true y (last layer).
  * Self-loops never touch the edge stream: a transposed copy of the
    core's own slice t_l^T lives in SBUF (built layer-by-layer via a
    PE transpose of each output tile) and seeds the aggregation.
  * Edges whose SOURCE is owned by the computing core ("own" stream,
    1/8 of edges) gather from the core-local slice xs_{l-1} instead of
    the AllGathered table, so they have no dependency on the previous
    AllGather and execute concurrently with it, hiding part of the
    collective.
  * Node v lives at permuted row prow = (v%8)*NPC + v//8 in every
    table including the host-prepared layer-0 table, so one edge
    layout/metadata set serves all three layers.

Per-edge mechanics (both streams): edges grouped into 128-edge chunks
sharing a dst tile; chunks gathered 1024 indices per dma_gather from
32768-row buckets (int16 index range); VectorE builds one-hot
S^T[e,d] = (iota[d] == dst_local[e]) in bf16; TensorE accumulates
aggT[f,d] += msg^T @ S^T in PSUM.  Remainders (<128 edges per
(tile,bucket)) are merged across the whole window (GSG supergroups) to
cut padding; per-(tile, merged-chunk) overlaps become separate matmul
ops whose dst metadata (sentinel 999) masks foreign edges.
"""

import numpy as np

# ----------------------------------------------------------------- config

FULL_CFG = dict(
    N=100000,          # nodes
    D=128,             # feature dim (= hidden)
    CORES=8,
    TPG=4,             # dst tiles per supergroup (PSUM bank = 512 fp32)
    GSG=2,             # supergroups per window (remainder-merge scope)
    BUCKET=32768,      # source rows per gather bucket (int16 idx range)
    GIDX=1024,         # indices per dma_gather instruction
)


def _derive(cfg):
    c = dict(cfg)
    c["NPC"] = c["N"] // c["CORES"]            # nodes per core
    assert c["NPC"] * c["CORES"] == c["N"]
    c["NT"] = (c["NPC"] + 127) // 128          # dst tiles per core
    c["NSG"] = (c["NT"] + c["TPG"] - 1) // c["TPG"]
    c["NW"] = (c["NSG"] + c["GSG"] - 1) // c["GSG"]
    c["NB"] = (c["N"] + c["BUCKET"] - 1) // c["BUCKET"]
    return c


# ----------------------------------------------------- host preprocessing

def _stream_layout(cfg, NB, dst_core, tile, bucket):
    """SPMD slot/op layout for one edge stream.

    Slot order: window-major, then bucket; within a bucket, each tile's
    FULL 128-edge chunks first (tiles in sg order), then one merged
    region packing all window tiles' remainders.  One gather range per
    (window, bucket).
    """
    CORES, NT, TPG, GSG = cfg["CORES"], cfg["NT"], cfg["TPG"], cfg["GSG"]
    NSG = cfg["NSG"]
    key = (dst_core * NT * NB + tile * NB + bucket).astype(np.int64)
    counts = np.bincount(key, minlength=CORES * NT * NB)
    counts = counts.reshape(CORES, NT, NB)
    mx = counts.max(axis=0)                     # [NT, NB]
    full = mx // 128
    rem = mx % 128

    full_slot0 = np.zeros((NT, NB), np.int64)
    full_op0 = np.zeros((NT, NB), np.int64)
    full128 = (full * 128).astype(np.int64)
    rem_pos0 = np.zeros((NT, NB), np.int64)
    rem_slot0 = np.zeros((NT, NB), np.int64)
    rem_op0 = np.zeros((NT, NB), np.int64)

    wins = [list(range(w * GSG, min((w + 1) * GSG, NSG)))
            for w in range(cfg["NW"])]
    winfo = []
    goff = 0          # global slot offset
    ooff = 0          # global op-column offset
    for w, sgs_idx in enumerate(wins):
        gathers = []                      # (bucket, s0_local, n_chunks)
        sgs = [dict(sg=sg,
                    tiles=list(range(sg * TPG, min((sg + 1) * TPG, NT))),
                    tile_ops={}) for sg in sgs_idx]
        for s in sgs:
            s["tile_ops"] = {t: [] for t in s["tiles"]}
        off = 0
        opo = 0
        for b in range(NB):
            s0 = off
            for s in sgs:
                for t in s["tiles"]:
                    f = int(full[t, b])
                    full_slot0[t, b] = goff + off
                    full_op0[t, b] = ooff + opo
                    for j in range(f):
                        s["tile_ops"][t].append((off + j, opo + j))
                    off += f
                    opo += f
            # merged remainder region across the whole window
            cum = 0
            mslot0 = off
            for s in sgs:
                for t in s["tiles"]:
                    r = int(rem[t, b])
                    if r == 0:
                        continue
                    rem_pos0[t, b] = cum
                    rem_slot0[t, b] = goff + mslot0
                    rem_op0[t, b] = ooff + opo
                    first, last = cum // 128, (cum + r - 1) // 128
                    for m in range(first, last + 1):
                        s["tile_ops"][t].append((mslot0 + m, opo + (m - first)))
                    opo += last - first + 1
                    cum += r
            off += -(-cum // 128)
            if off > s0:
                gathers.append((b, s0, off - s0))
        winfo.append(dict(sgs=sgs, S_w=off, OPS_w=opo, goff=goff,
                          ops_off=ooff, gathers=gathers))
        goff += off
        ooff += opo
    return dict(winfo=winfo, tot_slots=goff, tot_ops=ooff, key=key,
                full_slot0=full_slot0, full_op0=full_op0, full128=full128,
                rem_pos0=rem_pos0, rem_slot0=rem_slot0, rem_op0=rem_op0)


def _stream_blobs(cfg, lay, NB, rows_inb, dst_core, tile, bucket, dst_inb):
    """Per-core idx/dst blobs for one stream.

    idx16: [CORES, 128, tot*8] int16 in the SWDGE wrapped layout: global
    gather position i (slot s = i//128, partition p = i%128) lives at
    [p%16 (+16k replicas), s*8 + p//16].
    dstm: [CORES, 128, tot_ops] fp32, 999.0 on pad partitions so the
    is_equal one-hot build masks them.
    """
    CORES = cfg["CORES"]
    tot = lay["tot_slots"]
    tot_ops = lay["tot_ops"]
    key = lay["key"]

    order = np.argsort(key, kind="stable")
    counts_flat = np.bincount(key, minlength=CORES * cfg["NT"] * NB)
    seg_off = np.concatenate([[0], np.cumsum(counts_flat)])
    rank_sorted = np.arange(len(order)) - seg_off[key[order]]
    rank = np.empty(len(order), np.int64)
    rank[order] = rank_sorted

    f128 = lay["full128"][tile, bucket]
    is_full = rank < f128
    pos = lay["rem_pos0"][tile, bucket] + (rank - f128)
    gslot = np.where(is_full,
                     lay["full_slot0"][tile, bucket] + rank // 128,
                     lay["rem_slot0"][tile, bucket] + pos // 128)
    part = np.where(is_full, rank % 128, pos % 128)
    opcol = np.where(is_full,
                     lay["full_op0"][tile, bucket] + rank // 128,
                     lay["rem_op0"][tile, bucket]
                     + pos // 128 - lay["rem_pos0"][tile, bucket] // 128)

    idx16 = np.zeros((CORES, 16, tot * 8), np.int16)
    dstm = np.full((CORES, 128, tot_ops), 999.0, np.float32)
    idx16[dst_core, part % 16, gslot * 8 + part // 16] = rows_inb.astype(
        np.int16)
    dstm[dst_core, part, opcol] = dst_inb.astype(np.float32)
    idx128 = np.tile(idx16, (1, 8, 1))          # replicate across Q7 cores
    return idx128, dstm


def preprocess(cfg, edge_index):
    N, CORES, NPC, NT = cfg["N"], cfg["CORES"], cfg["NPC"], cfg["NT"]
    BUCKET, NB = cfg["BUCKET"], cfg["NB"]
    ei = np.asarray(edge_index).astype(np.int64)
    src, dst = ei[0], ei[1]
    deg = (np.bincount(dst, minlength=N) + 1.0).astype(np.float64)
    dinv = 1.0 / np.sqrt(deg)

    # balanced node -> (core, local row) assignment: snake-deal nodes in
    # in-degree order across the CORES*NT (core, tile) bins so every
    # core's tile t sees a near-identical in-degree profile -- this is
    # what keeps the SPMD max-over-cores slot layout tight.
    order = np.argsort(-deg, kind="stable")
    nbins = CORES * NT
    binid = np.empty(N, np.int64)
    pos_in_bin = np.empty(N, np.int64)
    idx = np.arange(N)
    rnd = idx // nbins            # deal round
    col = idx % nbins
    snake = np.where(rnd % 2 == 0, col, nbins - 1 - col)
    binid[order] = snake
    pos_in_bin[order] = rnd
    own_core = binid % CORES
    tile_of = binid // CORES
    loc_v = tile_of * 128 + pos_in_bin
    keep = loc_v < NPC            # last tile is short (NPC % 128 != 0)
    # nodes dealt past NPC wrap into earlier free slots of their core
    for c in range(CORES):
        mc = own_core == c
        over = mc & ~keep
        if over.any():
            used = np.zeros(NPC, bool)
            used[loc_v[mc & keep]] = True
            free = np.flatnonzero(~used)
            loc_v[over] = free[:over.sum()]
    prow_v = own_core * NPC + loc_v

    oc = lambda v: own_core[v]
    loc = lambda v: loc_v[v]
    prow = prow_v[src]

    dst_core = oc(dst)
    dst_local = loc(dst)
    tile = dst_local // 128
    dst_inb = dst_local % 128

    own = oc(src) == dst_core
    # The main stream gathers from the (core-major) AllGather table, so a
    # bucket never contains the gathering core's own rows symmetrically:
    # cores whose slice lies outside bucket b systematically see more
    # bucket-b edges than in-bucket cores, and the SPMD max-over-cores
    # layout would pad the difference.  Use own-source edges as a
    # balancing valve: move just enough of them into the main stream to
    # lift every (core, tile, bucket) cell to the cross-edge max; only
    # the surplus stays in the AllGather-independent own stream.
    bsrc = prow // BUCKET
    cellk = (dst_core * NT + tile) * NB + bsrc
    ncell = CORES * NT * NB
    cnt_x = np.bincount(cellk[~own], minlength=ncell).reshape(
        CORES, NT, NB)
    target = cnt_x.max(axis=0)                  # [NT, NB]
    deficit = (target[None] - cnt_x).reshape(-1)
    eo = np.flatnonzero(own)
    ko = cellk[eo]
    o_order = np.argsort(ko, kind="stable")
    o_cnt = np.bincount(ko, minlength=ncell)
    o_off = np.concatenate([[0], np.cumsum(o_cnt)])
    o_rank = np.empty(len(eo), np.int64)
    o_rank[o_order] = np.arange(len(eo)) - o_off[ko[o_order]]
    moved = o_rank < deficit[ko]
    own[eo[moved]] = False

    m = ~own
    lay_m = _stream_layout(cfg, NB, dst_core[m], tile[m], prow[m] // BUCKET)
    blob_m = _stream_blobs(cfg, lay_m, NB, prow[m] % BUCKET, dst_core[m],
                           tile[m], prow[m] // BUCKET, dst_inb[m])
    # own stream: same-core sources, gathered from the local slice
    z = np.zeros(own.sum(), np.int64)
    lay_o = _stream_layout(cfg, 1, dst_core[own], tile[own], z)
    blob_o = _stream_blobs(cfg, lay_o, 1, loc(src[own]), dst_core[own],
                           tile[own], z, dst_inb[own])
    return lay_m, lay_o, blob_m, blob_o, dinv, own_core, loc_v


# -------------------------------------------------------- device program

def build_program(cfg, lay_m, lay_o, n_layers=3, use_collective=True):
    import concourse.bass as bass  # noqa: F401
    import concourse.bacc as bacc
    import concourse.tile as tile
    import concourse.mybir as mybir

    f32 = mybir.dt.float32
    bf16 = mybir.dt.bfloat16
    i16 = mybir.dt.int16
    N, D, CORES = cfg["N"], cfg["D"], cfg["CORES"]
    NPC, NT, TPG = cfg["NPC"], cfg["NT"], cfg["TPG"]
    NB, BUCKET, GIDX = cfg["NB"], cfg["BUCKET"], cfg["GIDX"]
    GS = GIDX // 128                   # slots per gather instruction

    nc = bacc.Bacc("TRN2", target_bir_lowering=False, debug=False,
                   num_devices=CORES)

    x0p = nc.dram_tensor("x0p", [N, D], bf16, kind="ExternalInput")
    x0own = nc.dram_tensor("x0own", [NPC, D], bf16, kind="ExternalInput")
    x0T = nc.dram_tensor("x0T", [128, NT * 128], bf16, kind="ExternalInput")
    iota_in = nc.dram_tensor("iota", [128, 128], bf16, kind="ExternalInput")
    ident_in = nc.dram_tensor("ident", [128, 128], bf16,
                              kind="ExternalInput")
    W_in = [nc.dram_tensor(f"W{l}", [D, D], bf16, kind="ExternalInput")
            for l in range(3)]
    B_in = [nc.dram_tensor(f"b{l}", [1, D], bf16, kind="ExternalInput")
            for l in range(3)]
    invd_in = nc.dram_tensor("invd", [1, NT * 128], bf16,
                             kind="ExternalInput")
    dinvT_in = nc.dram_tensor("dinvT", [128, NT], f32, kind="ExternalInput")
    dinv2T_in = nc.dram_tensor("dinv2T", [128, NT], f32,
                               kind="ExternalInput")
    lays = {"m": lay_m, "o": lay_o}
    idx_in = {k: nc.dram_tensor(f"idx_{k}", [128, lays[k]["tot_slots"] * 8],
                                i16, kind="ExternalInput") for k in lays}
    dst_in = {k: nc.dram_tensor(f"dst_{k}", [128, lays[k]["tot_ops"]],
                                f32, kind="ExternalInput") for k in lays}
    y_out = nc.dram_tensor("y_out", [NPC, D], f32, kind="ExternalOutput")

    xs = [nc.dram_tensor(f"xslice{l}", [NPC, D], bf16) for l in range(2)]
    xg = [nc.dram_tensor(f"xgath{l}", [N, D], bf16, addr_space="Shared")
          for l in range(2)]

    with tile.TileContext(nc) as tc:
        with (
            tc.tile_pool(name="const", bufs=1) as constp,
            tc.tile_pool(name="msgm", bufs=3) as msgmp,
            tc.tile_pool(name="msgo", bufs=2) as msgop,
            tc.tile_pool(name="meta", bufs=3) as metap,
            tc.tile_pool(name="st", bufs=24) as stp,
            tc.tile_pool(name="sb2", bufs=7) as sb2p,
            tc.tile_pool(name="xsT", bufs=1) as xstp,
            tc.tile_pool(name="aggo", bufs=1) as aggop,
            tc.tile_pool(name="psA", bufs=3, space="PSUM") as psAp,
            tc.tile_pool(name="psY", bufs=3, space="PSUM") as psYp,
            tc.tile_pool(name="psT", bufs=2, space="PSUM") as psTp,
        ):
            iota_sb = constp.tile([128, 128], bf16, tag="iota")
            nc.sync.dma_start(iota_sb[:], iota_in[:, :])
            ident_sb = constp.tile([128, 128], bf16, tag="ident")
            nc.sync.dma_start(ident_sb[:], ident_in[:, :])
            dinvT = constp.tile([128, NT], f32, tag="dinvT")
            nc.sync.dma_start(dinvT[:], dinvT_in[:, :])
            dinv2T = constp.tile([128, NT], f32, tag="dinv2T")
            nc.sync.dma_start(dinv2T[:], dinv2T_in[:, :])
            W_sb, B_sb = [], []
            for l in range(3):
                w = constp.tile([128, 128], bf16, tag=f"W{l}")
                nc.sync.dma_start(w[:], W_in[l][:, :])
                W_sb.append(w)
                b = constp.tile([1, 128], bf16, tag=f"b{l}")
                nc.sync.dma_start(b[:], B_in[l][:, :])
                B_sb.append(b)

            # t_l^T for the core's own slice (self-loop seed + next layer)
            xsT = xstp.tile([128, NT * 128], bf16, tag="xsT")
            nc.sync.dma_start(xsT[:], x0T[:, :])

            def stream_ops(lay, key, x_src, bucket_rows, aggdst, psrc_merge):
                """Emit gathers + one-hot matmul ops for one stream/layer.

                aggdst(sg_dict, psA) is called once per supergroup after
                its accumulation completes.
                """
                for wi in lay["winfo"]:
                    if wi["S_w"] == 0:
                        continue
                    S_w = wi["S_w"]
                    OPS_w = wi["OPS_w"]
                    goff = wi["goff"]
                    ooff = wi["ops_off"]
                    pool = msgmp if key == "m" else msgop
                    msgb = pool.tile([128, S_w * 128], bf16, tag=f"msg{key}")
                    msgb3 = msgb[:].rearrange("p (s e) -> p s e", e=128)
                    idxt = metap.tile([128, S_w * 8], i16, tag=f"idx{key}")
                    dstt = metap.tile([128, OPS_w], f32, tag=f"dst{key}")
                    nc.sync.dma_start(idxt[:, :],
                                      idx_in[key][:, goff * 8:(goff + S_w) * 8])
                    nc.sync.dma_start(dstt[:, :],
                                      dst_in[key][:, ooff:ooff + OPS_w])
                    wctx = None
                    if key == "m":
                        t0w = wi["sgs"][0]["sg"] * TPG
                        t1w = wi["sgs"][-1]["tiles"][-1] + 1
                        invd_w = metap.tile([1, (t1w - t0w) * 128], bf16,
                                            tag="invd")
                        nc.sync.dma_start(
                            invd_w[:],
                            invd_in[:, t0w * 128:t1w * 128])
                        wctx = (invd_w, t0w)
                    for (b, g0, gch) in wi["gathers"]:
                        base = b * bucket_rows
                        rows = min(bucket_rows, x_src.shape[0] - base)
                        for s0 in range(g0, g0 + gch, GS):
                            nch = min(GS, g0 + gch - s0)
                            nidx = nch * 128
                            nc.gpsimd.dma_gather(
                                msgb3[:, s0:s0 + nch, :],
                                x_src[base:base + rows, :],
                                idxt[:, s0 * 8:(s0 + nch) * 8],
                                nidx, nidx, 128,
                            )
                    for sginfo in wi["sgs"]:
                        tiles = sginfo["tiles"]
                        ntl = len(tiles)
                        has_ops = any(sginfo["tile_ops"][t] for t in tiles)
                        if has_ops:
                            psA = psAp.tile([128, TPG * 128], f32, tag="psA")
                        else:
                            psA = None
                        for ti, t in enumerate(tiles):
                            ops = sginfo["tile_ops"][t]
                            for j, (s, oc_) in enumerate(ops):
                                stt = stp.tile([128, 128], bf16, tag="st")
                                nc.vector.tensor_scalar(
                                    stt[:], iota_sb[:],
                                    dstt[:, oc_:oc_ + 1], None,
                                    mybir.AluOpType.is_equal,
                                )
                                nc.tensor.matmul(
                                    psA[:, ti * 128:(ti + 1) * 128],
                                    msgb3[:, s, :], stt[:],
                                    start=(j == 0),
                                    stop=(j == len(ops) - 1),
                                )
                        aggdst(sginfo, psA, ntl, wctx)

            cnv = [0]

            def own_sink(sginfo, psA, ntl, wctx):
                # aggT_own[sg] = t_l^T[sg] + own-edge aggregate; tiles
                # with no ops in this stream never wrote their psA
                # columns, so they just copy the self-loop seed.
                sg = sginfo["sg"]
                for ti, t in enumerate(sginfo["tiles"]):
                    c = sg * TPG * 128 + ti * 128
                    if sginfo["tile_ops"][t]:
                        nc.vector.tensor_tensor(
                            aggT_own[:, c:c + 128],
                            psA[:, ti * 128:(ti + 1) * 128],
                            xsT[:, c:c + 128],
                            mybir.AluOpType.add)
                    else:
                        nc.vector.tensor_copy(
                            aggT_own[:, c:c + 128], xsT[:, c:c + 128])

            def main_sink(l, tgt, scaleT, build_T):
                def sink(sginfo, psA, ntl, wctx):
                    invd_w, t0w = wctx
                    sg = sginfo["sg"]
                    c0 = sg * TPG * 128
                    aggT = sb2p.tile([128, TPG * 128], bf16, tag="aggT")
                    if all(sginfo["tile_ops"][t] for t in sginfo["tiles"]):
                        nc.vector.tensor_tensor(
                            aggT[:, :ntl * 128], psA[:, :ntl * 128],
                            aggT_own[:, c0:c0 + ntl * 128],
                            mybir.AluOpType.add)
                    else:
                        for ti, t in enumerate(sginfo["tiles"]):
                            c = c0 + ti * 128
                            if sginfo["tile_ops"][t]:
                                nc.vector.tensor_tensor(
                                    aggT[:, ti * 128:(ti + 1) * 128],
                                    psA[:, ti * 128:(ti + 1) * 128],
                                    aggT_own[:, c:c + 128],
                                    mybir.AluOpType.add)
                            else:
                                nc.vector.tensor_copy(
                                    aggT[:, ti * 128:(ti + 1) * 128],
                                    aggT_own[:, c:c + 128])
                    for ti, t in enumerate(sginfo["tiles"]):
                        psY = psYp.tile([128, 128], f32, tag="psY")
                        nc.tensor.matmul(psY[:],
                                         aggT[:, ti * 128:(ti + 1) * 128],
                                         W_sb[l][:], start=True, stop=False)
                        nc.tensor.matmul(
                            psY[:],
                            invd_w[0:1, (t - t0w) * 128:(t - t0w + 1) * 128],
                            B_sb[l][:], start=False, stop=True)
                        ysb = sb2p.tile(
                            [128, 128], f32 if l == n_layers - 1 else bf16,
                            tag="ysb")
                        nc.scalar.mul(ysb[:], psY[:], scaleT[:, t:t + 1])
                        rows = min(128, NPC - t * 128)
                        nc.sync.dma_start(tgt[t * 128:t * 128 + rows, :],
                                          ysb[:rows, :])
                        if build_T:
                            psT = psTp.tile([128, 128], bf16, tag="psT")
                            nc.tensor.transpose(psT[:], ysb[:], ident_sb[:])
                            nc.scalar.copy(
                                xsT[:, t * 128:(t + 1) * 128], psT[:])
                return sink

            for l in range(n_layers):
                own_src = x0own if l == 0 else xs[l - 1]
                main_src = x0p if l == 0 else xg[l - 1]
                last = l == n_layers - 1
                tgt = y_out if last else xs[l]
                scaleT = dinvT if last else dinv2T
                aggT_own = aggop.tile([128, NT * 128], bf16, tag="aggo")
                # own stream first: no dependency on the previous AllGather
                stream_ops(lays["o"], "o", own_src, NPC, own_sink, None)
                stream_ops(lays["m"], "m", main_src, BUCKET,
                           main_sink(l, tgt, scaleT, not last), None)
                if not last and use_collective:
                    nc.gpsimd.collective_compute(
                        "AllGather",
                        mybir.AluOpType.bypass,
                        replica_groups=[list(range(CORES))],
                        ins=[xs[l][:, :].opt()],
                        outs=[xg[l][:, :].opt()],
                    )
    nc.compile()
    return nc


# ------------------------------------------------------------- execution

def make_in_maps(cfg, inputs, lay_m, lay_o, blob_m, blob_o, dinv,
                 own_core, loc_v):
    import ml_dtypes

    bf = ml_dtypes.bfloat16
    N, D, CORES, NPC, NT = (cfg["N"], cfg["D"], cfg["CORES"], cfg["NPC"],
                            cfg["NT"])
    idx_m, dst_m = blob_m
    idx_o, dst_o = blob_o
    iota = np.tile(np.arange(128, dtype=bf), (128, 1))
    ident = np.eye(128).astype(bf)

    x0 = np.asarray(inputs["node_features"], dtype=np.float64)
    x0s = x0 * dinv[:, None]
    # permuted table: row (v%8)*NPC + v//8 holds node v
    prow_v = own_core * NPC + loc_v
    x0p = np.empty((N, D), bf)
    x0p[prow_v] = x0s.astype(bf)
    dinvp = np.empty(N, np.float64)
    dinvp[prow_v] = dinv

    in_maps = []
    for c in range(CORES):
        sl = slice(c * NPC, (c + 1) * NPC)
        x0own = np.ascontiguousarray(x0p[sl])
        x0T = np.zeros((128, NT * 128), bf)
        x0T[:, :NPC] = x0own.T
        dv = dinvp[sl]
        dvT = np.zeros((128, NT), np.float32)
        dv2T = np.zeros((128, NT), np.float32)
        dpad = np.zeros(NT * 128)
        dpad[:NPC] = dv
        dvT[:, :] = dpad.reshape(NT, 128).T
        dv2T[:, :] = (dpad ** 2).reshape(NT, 128).T
        invd = np.zeros((1, NT * 128), bf)
        invd[0, :NPC] = (1.0 / dv).astype(bf)
        m = {
            "x0p": x0p,
            "x0own": x0own,
            "x0T": x0T,
            "iota": iota,
            "ident": ident,
            "invd": invd,
            "dinvT": dvT,
            "dinv2T": dv2T,
            "idx_m": np.ascontiguousarray(idx_m[c]),
            "dst_m": np.ascontiguousarray(dst_m[c]),
            "idx_o": np.ascontiguousarray(idx_o[c]),
            "dst_o": np.ascontiguousarray(dst_o[c]),
        }
        for l in range(3):
            m[f"W{l}"] = np.asarray(inputs[f"W{l}"], dtype=np.float32).astype(
                bf)
            m[f"b{l}"] = np.asarray(inputs[f"b{l}"],
                                    dtype=np.float32).reshape(1, D).astype(bf)
        in_maps.append(m)
    return in_maps


def unshard_output(cfg, results, own_core, loc_v):
    N, D = cfg["N"], cfg["D"]
    out = np.empty((N, D), np.float32)
    for c in range(cfg["CORES"]):
        m = own_core == c
        out[m] = results[c]["y_out"][loc_v[m]]
    return out


_CACHE = {}


def kernel(**inputs) -> np.ndarray:
    import time

    cfg = _derive(FULL_CFG)
    ekey = hash(np.asarray(inputs["edge_index"]).tobytes())
    if ekey in _CACHE:
        pre, nc = _CACHE[ekey]
    else:
        pre = preprocess(cfg, inputs["edge_index"])
        nc = build_program(cfg, pre[0], pre[1])
        _CACHE.clear()
        _CACHE[ekey] = (pre, nc)
    in_maps = make_in_maps(cfg, inputs, *pre)
    own_core, loc_v = pre[5], pre[6]
    from concourse import bass_utils

    # The axon-tunneled device occasionally dies mid-run
    # (NRT_EXEC_UNIT_UNRECOVERABLE) and the worker restarts itself over
    # the next minute or two; retry a few times before giving up.
    last_exc = None
    for attempt, backoff_s in enumerate([0, 90, 180, 240]):
        if backoff_s:
            time.sleep(backoff_s)
        try:
            res = bass_utils.run_bass_kernel_spmd(
                nc, in_maps, core_ids=list(range(cfg["CORES"])))
            return unshard_output(cfg, res.results, own_core, loc_v)
        except Exception as exc:  # transient worker/device failures
            last_exc = exc
            try:
                import jax
                jax.clear_caches()
            except Exception:
                pass
    raise last_exc


# revision 21
# speedup vs baseline: 1.0194x; 1.0194x over previous
"""3-layer GCN on 8 Trainium2 NeuronCores (Bass/Tile).

Math (per layer, identical to PyG GCNConv with self-loops):
    x_{l+1} = A_hat @ (x_l @ W_l) + b_l,   A_hat = D^-1/2 (A+I) D^-1/2

Key restructurings vs the straightforward form:
  * Aggregate first, GEMM second (associativity): each core runs the
    128x128 GEMM only on its own 1/8 of the nodes.
  * Norm folding: the gather table stores t_l = D^-1/2 x_l, so the
    aggregation matrix is the 0/1 adjacency (S tiles need no norm
    column) and the D^-1/2 on the output side becomes a per-partition
    scale at the PSUM->SBUF copy.  The bias rides a rank-1 matmul
    invd (x) b so the same scale produces either t_{l+1} (layers 0,1)
    or the true y (last layer).
  * Self-loops never touch the edge stream: a transposed copy of the
    core's own slice t_l^T lives in SBUF (built layer-by-layer via a
    PE transpose of each output tile) and seeds the aggregation.
  * Edges whose SOURCE is owned by the computing core ("own" stream,
    1/8 of edges) gather from the core-local slice xs_{l-1} instead of
    the AllGathered table, so they have no dependency on the previous
    AllGather and execute concurrently with it, hiding part of the
    collective.
  * Node v lives at permuted row prow = (v%8)*NPC + v//8 in every
    table including the host-prepared layer-0 table, so one edge
    layout/metadata set serves all three layers.

Per-edge mechanics (both streams): edges grouped into 128-edge chunks
sharing a dst tile; chunks gathered 1024 indices per dma_gather from
32768-row buckets (int16 index range); VectorE builds one-hot
S^T[e,d] = (iota[d] == dst_local[e]) in bf16; TensorE accumulates
aggT[f,d] += msg^T @ S^T in PSUM.  Remainders (<128 edges per
(tile,bucket)) are merged across the whole window (GSG supergroups) to
cut padding; per-(tile, merged-chunk) overlaps become separate matmul
ops whose dst metadata (sentinel 999) masks foreign edges.
"""

import numpy as np

# ----------------------------------------------------------------- config

FULL_CFG = dict(
    N=100000,          # nodes
    D=128,             # feature dim (= hidden)
    CORES=8,
    TPG=4,             # dst tiles per supergroup (PSUM bank = 512 fp32)
    GSG=2,             # supergroups per window (remainder-merge scope)
    BUCKET=32768,      # source rows per gather bucket (int16 idx range)
    GIDX=1024,         # indices per dma_gather instruction
)


def _derive(cfg):
    c = dict(cfg)
    c["NPC"] = c["N"] // c["CORES"]            # nodes per core
    assert c["NPC"] * c["CORES"] == c["N"]
    c["NT"] = (c["NPC"] + 127) // 128          # dst tiles per core
    c["NSG"] = (c["NT"] + c["TPG"] - 1) // c["TPG"]
    c["NW"] = (c["NSG"] + c["GSG"] - 1) // c["GSG"]
    c["NB"] = (c["N"] + c["BUCKET"] - 1) // c["BUCKET"]
    return c


# ----------------------------------------------------- host preprocessing

def _stream_layout(cfg, NB, dst_core, tile, bucket):
    """SPMD slot/op layout for one edge stream.

    Slot order: window-major, then bucket; within a bucket, each tile's
    FULL 128-edge chunks first (tiles in sg order), then one merged
    region packing all window tiles' remainders.  One gather range per
    (window, bucket).
    """
    CORES, NT, TPG, GSG = cfg["CORES"], cfg["NT"], cfg["TPG"], cfg["GSG"]
    NSG = cfg["NSG"]
    key = (dst_core * NT * NB + tile * NB + bucket).astype(np.int64)
    counts = np.bincount(key, minlength=CORES * NT * NB)
    counts = counts.reshape(CORES, NT, NB)
    mx = counts.max(axis=0)                     # [NT, NB]
    full = mx // 128
    rem = mx % 128

    full_slot0 = np.zeros((NT, NB), np.int64)
    full_op0 = np.zeros((NT, NB), np.int64)
    full128 = (full * 128).astype(np.int64)
    rem_pos0 = np.zeros((NT, NB), np.int64)
    rem_slot0 = np.zeros((NT, NB), np.int64)
    rem_op0 = np.zeros((NT, NB), np.int64)

    wins = [list(range(w * GSG, min((w + 1) * GSG, NSG)))
            for w in range(cfg["NW"])]
    winfo = []
    goff = 0          # global slot offset
    ooff = 0          # global op-column offset
    for w, sgs_idx in enumerate(wins):
        gathers = []                      # (bucket, s0_local, n_chunks)
        sgs = [dict(sg=sg,
                    tiles=list(range(sg * TPG, min((sg + 1) * TPG, NT))),
                    tile_ops={}) for sg in sgs_idx]
        for s in sgs:
            s["tile_ops"] = {t: [] for t in s["tiles"]}
        off = 0
        opo = 0
        for b in range(NB):
            s0 = off
            for s in sgs:
                for t in s["tiles"]:
                    f = int(full[t, b])
                    full_slot0[t, b] = goff + off
                    full_op0[t, b] = ooff + opo
                    for j in range(f):
                        s["tile_ops"][t].append((off + j, opo + j))
                    off += f
                    opo += f
            # merged remainder region across the whole window
            cum = 0
            mslot0 = off
            for s in sgs:
                for t in s["tiles"]:
                    r = int(rem[t, b])
                    if r == 0:
                        continue
                    rem_pos0[t, b] = cum
                    rem_slot0[t, b] = goff + mslot0
                    rem_op0[t, b] = ooff + opo
                    first, last = cum // 128, (cum + r - 1) // 128
                    for m in range(first, last + 1):
                        s["tile_ops"][t].append((mslot0 + m, opo + (m - first)))
                    opo += last - first + 1
                    cum += r
            off += -(-cum // 128)
            if off > s0:
                gathers.append((b, s0, off - s0))
        winfo.append(dict(sgs=sgs, S_w=off, OPS_w=opo, goff=goff,
                          ops_off=ooff, gathers=gathers))
        goff += off
        ooff += opo
    return dict(winfo=winfo, tot_slots=goff, tot_ops=ooff, key=key,
                full_slot0=full_slot0, full_op0=full_op0, full128=full128,
                rem_pos0=rem_pos0, rem_slot0=rem_slot0, rem_op0=rem_op0)


def _stream_blobs(cfg, lay, NB, rows_inb, dst_core, tile, bucket, dst_inb):
    """Per-core idx/dst blobs for one stream.

    idx16: [CORES, 128, tot*8] int16 in the SWDGE wrapped layout: global
    gather position i (slot s = i//128, partition p = i%128) lives at
    [p%16 (+16k replicas), s*8 + p//16].
    dstm: [CORES, 128, tot_ops] fp32, 999.0 on pad partitions so the
    is_equal one-hot build masks them.
    """
    CORES = cfg["CORES"]
    tot = lay["tot_slots"]
    tot_ops = lay["tot_ops"]
    key = lay["key"]

    order = np.argsort(key, kind="stable")
    counts_flat = np.bincount(key, minlength=CORES * cfg["NT"] * NB)
    seg_off = np.concatenate([[0], np.cumsum(counts_flat)])
    rank_sorted = np.arange(len(order)) - seg_off[key[order]]
    rank = np.empty(len(order), np.int64)
    rank[order] = rank_sorted

    f128 = lay["full128"][tile, bucket]
    is_full = rank < f128
    pos = lay["rem_pos0"][tile, bucket] + (rank - f128)
    gslot = np.where(is_full,
                     lay["full_slot0"][tile, bucket] + rank // 128,
                     lay["rem_slot0"][tile, bucket] + pos // 128)
    part = np.where(is_full, rank % 128, pos % 128)
    opcol = np.where(is_full,
                     lay["full_op0"][tile, bucket] + rank // 128,
                     lay["rem_op0"][tile, bucket]
                     + pos // 128 - lay["rem_pos0"][tile, bucket] // 128)

    idx16 = np.zeros((CORES, 16, tot * 8), np.int16)
    dstm = np.full((CORES, 128, tot_ops), 999.0, np.float32)
    idx16[dst_core, part % 16, gslot * 8 + part // 16] = rows_inb.astype(
        np.int16)
    dstm[dst_core, part, opcol] = dst_inb.astype(np.float32)
    idx128 = np.tile(idx16, (1, 8, 1))          # replicate across Q7 cores
    return idx128, dstm


def preprocess(cfg, edge_index):
    N, CORES, NPC, NT = cfg["N"], cfg["CORES"], cfg["NPC"], cfg["NT"]
    BUCKET, NB = cfg["BUCKET"], cfg["NB"]
    ei = np.asarray(edge_index).astype(np.int64)
    src, dst = ei[0], ei[1]
    deg = (np.bincount(dst, minlength=N) + 1.0).astype(np.float64)
    dinv = 1.0 / np.sqrt(deg)

    # balanced node -> (core, local row) assignment: snake-deal nodes in
    # in-degree order across the CORES*NT (core, tile) bins so every
    # core's tile t sees a near-identical in-degree profile -- this is
    # what keeps the SPMD max-over-cores slot layout tight.
    order = np.argsort(-deg, kind="stable")
    nbins = CORES * NT
    binid = np.empty(N, np.int64)
    pos_in_bin = np.empty(N, np.int64)
    idx = np.arange(N)
    rnd = idx // nbins            # deal round
    col = idx % nbins
    snake = np.where(rnd % 2 == 0, col, nbins - 1 - col)
    binid[order] = snake
    pos_in_bin[order] = rnd
    own_core = binid % CORES
    tile_of = binid // CORES
    loc_v = tile_of * 128 + pos_in_bin
    keep = loc_v < NPC            # last tile is short (NPC % 128 != 0)
    # nodes dealt past NPC wrap into earlier free slots of their core
    for c in range(CORES):
        mc = own_core == c
        over = mc & ~keep
        if over.any():
            used = np.zeros(NPC, bool)
            used[loc_v[mc & keep]] = True
            free = np.flatnonzero(~used)
            loc_v[over] = free[:over.sum()]
    prow_v = own_core * NPC + loc_v

    oc = lambda v: own_core[v]
    loc = lambda v: loc_v[v]
    prow = prow_v[src]

    dst_core = oc(dst)
    dst_local = loc(dst)
    tile = dst_local // 128
    dst_inb = dst_local % 128

    own = oc(src) == dst_core
    # The main stream gathers from the (core-major) AllGather table, so a
    # bucket never contains the gathering core's own rows symmetrically:
    # cores whose slice lies outside bucket b systematically see more
    # bucket-b edges than in-bucket cores, and the SPMD max-over-cores
    # layout would pad the difference.  Use own-source edges as a
    # balancing valve: move just enough of them into the main stream to
    # lift every (core, tile, bucket) cell to the cross-edge max; only
    # the surplus stays in the AllGather-independent own stream.
    bsrc = prow // BUCKET
    cellk = (dst_core * NT + tile) * NB + bsrc
    ncell = CORES * NT * NB
    cnt_x = np.bincount(cellk[~own], minlength=ncell).reshape(
        CORES, NT, NB)
    target = cnt_x.max(axis=0)                  # [NT, NB]
    deficit = (target[None] - cnt_x).reshape(-1)
    eo = np.flatnonzero(own)
    ko = cellk[eo]
    o_order = np.argsort(ko, kind="stable")
    o_cnt = np.bincount(ko, minlength=ncell)
    o_off = np.concatenate([[0], np.cumsum(o_cnt)])
    o_rank = np.empty(len(eo), np.int64)
    o_rank[o_order] = np.arange(len(eo)) - o_off[ko[o_order]]
    moved = o_rank < deficit[ko]
    own[eo[moved]] = False

    m = ~own
    lay_m = _stream_layout(cfg, NB, dst_core[m], tile[m], prow[m] // BUCKET)
    blob_m = _stream_blobs(cfg, lay_m, NB, prow[m] % BUCKET, dst_core[m],
                           tile[m], prow[m] // BUCKET, dst_inb[m])
    # own stream: same-core sources, gathered from the local slice
    z = np.zeros(own.sum(), np.int64)
    lay_o = _stream_layout(cfg, 1, dst_core[own], tile[own], z)
    blob_o = _stream_blobs(cfg, lay_o, 1, loc(src[own]), dst_core[own],
                           tile[own], z, dst_inb[own])
    return lay_m, lay_o, blob_m, blob_o, dinv, own_core, loc_v


# -------------------------------------------------------- device program

def build_program(cfg, lay_m, lay_o, n_layers=3, use_collective=True):
    import concourse.bass as bass  # noqa: F401
    import concourse.bacc as bacc
    import concourse.tile as tile
    import concourse.mybir as mybir

    f32 = mybir.dt.float32
    bf16 = mybir.dt.bfloat16
    i16 = mybir.dt.int16
    N, D, CORES = cfg["N"], cfg["D"], cfg["CORES"]
    NPC, NT, TPG = cfg["NPC"], cfg["NT"], cfg["TPG"]
    NB, BUCKET, GIDX = cfg["NB"], cfg["BUCKET"], cfg["GIDX"]
    GS = GIDX // 128                   # slots per gather instruction

    nc = bacc.Bacc("TRN2", target_bir_lowering=False, debug=False,
                   num_devices=CORES)

    x0p = nc.dram_tensor("x0p", [N, D], bf16, kind="ExternalInput")
    x0own = nc.dram_tensor("x0own", [NPC, D], bf16, kind="ExternalInput")
    x0T = nc.dram_tensor("x0T", [128, NT * 128], bf16, kind="ExternalInput")
    iota_in = nc.dram_tensor("iota", [128, 128], bf16, kind="ExternalInput")
    ident_in = nc.dram_tensor("ident", [128, 128], bf16,
                              kind="ExternalInput")
    W_in = [nc.dram_tensor(f"W{l}", [D, D], bf16, kind="ExternalInput")
            for l in range(3)]
    B_in = [nc.dram_tensor(f"b{l}", [1, D], bf16, kind="ExternalInput")
            for l in range(3)]
    invd_in = nc.dram_tensor("invd", [1, NT * 128], bf16,
                             kind="ExternalInput")
    dinvT_in = nc.dram_tensor("dinvT", [128, NT], f32, kind="ExternalInput")
    dinv2T_in = nc.dram_tensor("dinv2T", [128, NT], f32,
                               kind="ExternalInput")
    lays = {"m": lay_m, "o": lay_o}
    idx_in = {k: nc.dram_tensor(f"idx_{k}", [128, lays[k]["tot_slots"] * 8],
                                i16, kind="ExternalInput") for k in lays}
    dst_in = {k: nc.dram_tensor(f"dst_{k}", [128, lays[k]["tot_ops"]],
                                f32, kind="ExternalInput") for k in lays}
    y_out = nc.dram_tensor("y_out", [NPC, D], f32, kind="ExternalOutput")

    xs = [nc.dram_tensor(f"xslice{l}", [NPC, D], bf16) for l in range(2)]
    xg = [nc.dram_tensor(f"xgath{l}", [N, D], bf16, addr_space="Shared")
          for l in range(2)]

    with tile.TileContext(nc) as tc:
        with (
            tc.tile_pool(name="const", bufs=1) as constp,
            tc.tile_pool(name="msgm", bufs=3) as msgmp,
            tc.tile_pool(name="msgo", bufs=2) as msgop,
            tc.tile_pool(name="meta", bufs=3) as metap,
            tc.tile_pool(name="st", bufs=24) as stp,
            tc.tile_pool(name="sb2", bufs=12) as sb2p,
            tc.tile_pool(name="xsT", bufs=1) as xstp,
            tc.tile_pool(name="aggo", bufs=1) as aggop,
            tc.tile_pool(name="psA", bufs=3, space="PSUM") as psAp,
            tc.tile_pool(name="psY", bufs=3, space="PSUM") as psYp,
            tc.tile_pool(name="psT", bufs=2, space="PSUM") as psTp,
        ):
            iota_sb = constp.tile([128, 128], bf16, tag="iota")
            nc.sync.dma_start(iota_sb[:], iota_in[:, :])
            ident_sb = constp.tile([128, 128], bf16, tag="ident")
            nc.sync.dma_start(ident_sb[:], ident_in[:, :])
            dinvT = constp.tile([128, NT], f32, tag="dinvT")
            nc.sync.dma_start(dinvT[:], dinvT_in[:, :])
            dinv2T = constp.tile([128, NT], f32, tag="dinv2T")
            nc.sync.dma_start(dinv2T[:], dinv2T_in[:, :])
            W_sb, B_sb = [], []
            for l in range(3):
                w = constp.tile([128, 128], bf16, tag=f"W{l}")
                nc.sync.dma_start(w[:], W_in[l][:, :])
                W_sb.append(w)
                b = constp.tile([1, 128], bf16, tag=f"b{l}")
                nc.sync.dma_start(b[:], B_in[l][:, :])
                B_sb.append(b)

            # t_l^T for the core's own slice (self-loop seed + next layer)
            xsT = xstp.tile([128, NT * 128], bf16, tag="xsT")
            nc.sync.dma_start(xsT[:], x0T[:, :])

            def stream_ops(lay, key, x_src, bucket_rows, aggdst, psrc_merge):
                """Emit gathers + one-hot matmul ops for one stream/layer.

                aggdst(sg_dict, psA) is called once per supergroup after
                its accumulation completes.
                """
                for wi in lay["winfo"]:
                    if wi["S_w"] == 0:
                        continue
                    S_w = wi["S_w"]
                    OPS_w = wi["OPS_w"]
                    goff = wi["goff"]
                    ooff = wi["ops_off"]
                    pool = msgmp if key == "m" else msgop
                    msgb = pool.tile([128, S_w * 128], bf16, tag=f"msg{key}")
                    msgb3 = msgb[:].rearrange("p (s e) -> p s e", e=128)
                    idxt = metap.tile([128, S_w * 8], i16, tag=f"idx{key}")
                    dstt = metap.tile([128, OPS_w], f32, tag=f"dst{key}")
                    nc.sync.dma_start(idxt[:, :],
                                      idx_in[key][:, goff * 8:(goff + S_w) * 8])
                    nc.sync.dma_start(dstt[:, :],
                                      dst_in[key][:, ooff:ooff + OPS_w])
                    wctx = None
                    if key == "m":
                        t0w = wi["sgs"][0]["sg"] * TPG
                        t1w = wi["sgs"][-1]["tiles"][-1] + 1
                        invd_w = metap.tile([1, (t1w - t0w) * 128], bf16,
                                            tag="invd")
                        nc.sync.dma_start(
                            invd_w[:],
                            invd_in[:, t0w * 128:t1w * 128])
                        wctx = (invd_w, t0w)
                    for (b, g0, gch) in wi["gathers"]:
                        base = b * bucket_rows
                        rows = min(bucket_rows, x_src.shape[0] - base)
                        for s0 in range(g0, g0 + gch, GS):
                            nch = min(GS, g0 + gch - s0)
                            nidx = nch * 128
                            nc.gpsimd.dma_gather(
                                msgb3[:, s0:s0 + nch, :],
                                x_src[base:base + rows, :],
                                idxt[:, s0 * 8:(s0 + nch) * 8],
                                nidx, nidx, 128,
                            )
                    for sginfo in wi["sgs"]:
                        tiles = sginfo["tiles"]
                        ntl = len(tiles)
                        has_ops = any(sginfo["tile_ops"][t] for t in tiles)
                        if has_ops:
                            psA = psAp.tile([128, TPG * 128], f32, tag="psA")
                        else:
                            psA = None
                        for ti, t in enumerate(tiles):
                            ops = sginfo["tile_ops"][t]
                            for j, (s, oc_) in enumerate(ops):
                                stt = stp.tile([128, 128], bf16, tag="st")
                                nc.vector.tensor_scalar(
                                    stt[:], iota_sb[:],
                                    dstt[:, oc_:oc_ + 1], None,
                                    mybir.AluOpType.is_equal,
                                )
                                nc.tensor.matmul(
                                    psA[:, ti * 128:(ti + 1) * 128],
                                    msgb3[:, s, :], stt[:],
                                    start=(j == 0),
                                    stop=(j == len(ops) - 1),
                                )
                        aggdst(sginfo, psA, ntl, wctx)

            cnv = [0]

            def own_sink(sginfo, psA, ntl, wctx):
                # aggT_own[sg] = t_l^T[sg] + own-edge aggregate; tiles
                # with no ops in this stream never wrote their psA
                # columns, so they just copy the self-loop seed.
                sg = sginfo["sg"]
                for ti, t in enumerate(sginfo["tiles"]):
                    c = sg * TPG * 128 + ti * 128
                    if sginfo["tile_ops"][t]:
                        nc.vector.tensor_tensor(
                            aggT_own[:, c:c + 128],
                            psA[:, ti * 128:(ti + 1) * 128],
                            xsT[:, c:c + 128],
                            mybir.AluOpType.add)
                    else:
                        nc.vector.tensor_copy(
                            aggT_own[:, c:c + 128], xsT[:, c:c + 128])

            def main_sink(l, tgt, scaleT, build_T):
                def sink(sginfo, psA, ntl, wctx):
                    invd_w, t0w = wctx
                    sg = sginfo["sg"]
                    c0 = sg * TPG * 128
                    aggT = sb2p.tile([128, TPG * 128], bf16, tag="aggT")
                    if all(sginfo["tile_ops"][t] for t in sginfo["tiles"]):
                        nc.vector.tensor_tensor(
                            aggT[:, :ntl * 128], psA[:, :ntl * 128],
                            aggT_own[:, c0:c0 + ntl * 128],
                            mybir.AluOpType.add)
                    else:
                        for ti, t in enumerate(sginfo["tiles"]):
                            c = c0 + ti * 128
                            if sginfo["tile_ops"][t]:
                                nc.vector.tensor_tensor(
                                    aggT[:, ti * 128:(ti + 1) * 128],
                                    psA[:, ti * 128:(ti + 1) * 128],
                                    aggT_own[:, c:c + 128],
                                    mybir.AluOpType.add)
                            else:
                                nc.vector.tensor_copy(
                                    aggT[:, ti * 128:(ti + 1) * 128],
                                    aggT_own[:, c:c + 128])
                    for ti, t in enumerate(sginfo["tiles"]):
                        psY = psYp.tile([128, 128], f32, tag="psY")
                        nc.tensor.matmul(psY[:],
                                         aggT[:, ti * 128:(ti + 1) * 128],
                                         W_sb[l][:], start=True, stop=False)
                        nc.tensor.matmul(
                            psY[:],
                            invd_w[0:1, (t - t0w) * 128:(t - t0w + 1) * 128],
                            B_sb[l][:], start=False, stop=True)
                        ysb = sb2p.tile(
                            [128, 128], f32 if l == n_layers - 1 else bf16,
                            tag="ysb")
                        nc.scalar.mul(ysb[:], psY[:], scaleT[:, t:t + 1])
                        rows = min(128, NPC - t * 128)
                        nc.sync.dma_start(tgt[t * 128:t * 128 + rows, :],
                                          ysb[:rows, :])
                        if build_T:
                            psT = psTp.tile([128, 128], bf16, tag="psT")
                            nc.tensor.transpose(psT[:], ysb[:], ident_sb[:])
                            nc.scalar.copy(
                                xsT[:, t * 128:(t + 1) * 128], psT[:])
                return sink

            for l in range(n_layers):
                own_src = x0own if l == 0 else xs[l - 1]
                main_src = x0p if l == 0 else xg[l - 1]
                last = l == n_layers - 1
                tgt = y_out if last else xs[l]
                scaleT = dinvT if last else dinv2T
                aggT_own = aggop.tile([128, NT * 128], bf16, tag="aggo")
                # own stream first: no dependency on the previous AllGather
                stream_ops(lays["o"], "o", own_src, NPC, own_sink, None)
                stream_ops(lays["m"], "m", main_src, BUCKET,
                           main_sink(l, tgt, scaleT, not last), None)
                if not last and use_collective:
                    nc.gpsimd.collective_compute(
                        "AllGather",
                        mybir.AluOpType.bypass,
                        replica_groups=[list(range(CORES))],
                        ins=[xs[l][:, :].opt()],
                        outs=[xg[l][:, :].opt()],
                    )
    nc.compile()
    return nc


# ------------------------------------------------------------- execution

def make_in_maps(cfg, inputs, lay_m, lay_o, blob_m, blob_o, dinv,
                 own_core, loc_v):
    import ml_dtypes

    bf = ml_dtypes.bfloat16
    N, D, CORES, NPC, NT = (cfg["N"], cfg["D"], cfg["CORES"], cfg["NPC"],
                            cfg["NT"])
    idx_m, dst_m = blob_m
    idx_o, dst_o = blob_o
    iota = np.tile(np.arange(128, dtype=bf), (128, 1))
    ident = np.eye(128).astype(bf)

    x0 = np.asarray(inputs["node_features"], dtype=np.float64)
    x0s = x0 * dinv[:, None]
    # permuted table: row (v%8)*NPC + v//8 holds node v
    prow_v = own_core * NPC + loc_v
    x0p = np.empty((N, D), bf)
    x0p[prow_v] = x0s.astype(bf)
    dinvp = np.empty(N, np.float64)
    dinvp[prow_v] = dinv

    in_maps = []
    for c in range(CORES):
        sl = slice(c * NPC, (c + 1) * NPC)
        x0own = np.ascontiguousarray(x0p[sl])
        x0T = np.zeros((128, NT * 128), bf)
        x0T[:, :NPC] = x0own.T
        dv = dinvp[sl]
        dvT = np.zeros((128, NT), np.float32)
        dv2T = np.zeros((128, NT), np.float32)
        dpad = np.zeros(NT * 128)
        dpad[:NPC] = dv
        dvT[:, :] = dpad.reshape(NT, 128).T
        dv2T[:, :] = (dpad ** 2).reshape(NT, 128).T
        invd = np.zeros((1, NT * 128), bf)
        invd[0, :NPC] = (1.0 / dv).astype(bf)
        m = {
            "x0p": x0p,
            "x0own": x0own,
            "x0T": x0T,
            "iota": iota,
            "ident": ident,
            "invd": invd,
            "dinvT": dvT,
            "dinv2T": dv2T,
            "idx_m": np.ascontiguousarray(idx_m[c]),
            "dst_m": np.ascontiguousarray(dst_m[c]),
            "idx_o": np.ascontiguousarray(idx_o[c]),
            "dst_o": np.ascontiguousarray(dst_o[c]),
        }
        for l in range(3):
            m[f"W{l}"] = np.asarray(inputs[f"W{l}"], dtype=np.float32).astype(
                bf)
            m[f"b{l}"] = np.asarray(inputs[f"b{l}"],
                                    dtype=np.float32).reshape(1, D).astype(bf)
        in_maps.append(m)
    return in_maps


def unshard_output(cfg, results, own_core, loc_v):
    N, D = cfg["N"], cfg["D"]
    out = np.empty((N, D), np.float32)
    for c in range(cfg["CORES"]):
        m = own_core == c
        out[m] = results[c]["y_out"][loc_v[m]]
    return out


_CACHE = {}


def kernel(**inputs) -> np.ndarray:
    import time

    cfg = _derive(FULL_CFG)
    ekey = hash(np.asarray(inputs["edge_index"]).tobytes())
    if ekey in _CACHE:
        pre, nc = _CACHE[ekey]
    else:
        pre = preprocess(cfg, inputs["edge_index"])
        nc = build_program(cfg, pre[0], pre[1])
        _CACHE.clear()
        _CACHE[ekey] = (pre, nc)
    in_maps = make_in_maps(cfg, inputs, *pre)
    own_core, loc_v = pre[5], pre[6]
    from concourse import bass_utils

    # The axon-tunneled device occasionally dies mid-run
    # (NRT_EXEC_UNIT_UNRECOVERABLE) and the worker restarts itself over
    # the next minute or two; retry a few times before giving up.
    last_exc = None
    for attempt, backoff_s in enumerate([0, 90, 180, 240]):
        if backoff_s:
            time.sleep(backoff_s)
        try:
            res = bass_utils.run_bass_kernel_spmd(
                nc, in_maps, core_ids=list(range(cfg["CORES"])))
            return unshard_output(cfg, res.results, own_core, loc_v)
        except Exception as exc:  # transient worker/device failures
            last_exc = exc
            try:
                import jax
                jax.clear_caches()
            except Exception:
                pass
    raise last_exc


# revision 25
# speedup vs baseline: 1.0363x; 1.0166x over previous
"""3-layer GCN on 8 Trainium2 NeuronCores (Bass/Tile).

Math (per layer, identical to PyG GCNConv with self-loops):
    x_{l+1} = A_hat @ (x_l @ W_l) + b_l,   A_hat = D^-1/2 (A+I) D^-1/2

Key restructurings vs the straightforward form:
  * Aggregate first, GEMM second (associativity): each core runs the
    128x128 GEMM only on its own 1/8 of the nodes.
  * Norm folding: the gather table stores t_l = D^-1/2 x_l, so the
    aggregation matrix is the 0/1 adjacency (S tiles need no norm
    column) and the D^-1/2 on the output side becomes a per-partition
    scale at the PSUM->SBUF copy.  The bias rides a rank-1 matmul
    invd (x) b so the same scale produces either t_{l+1} (layers 0,1)
    or the true y (last layer).
  * Self-loops never touch the edge stream: a transposed copy of the
    core's own slice t_l^T lives in SBUF (built layer-by-layer via a
    PE transpose of each output tile) and seeds the aggregation.
  * Edges whose SOURCE is owned by the computing core ("own" stream,
    1/8 of edges) gather from the core-local slice xs_{l-1} instead of
    the AllGathered table, so they have no dependency on the previous
    AllGather and execute concurrently with it, hiding part of the
    collective.
  * Node v lives at permuted row prow = (v%8)*NPC + v//8 in every
    table including the host-prepared layer-0 table, so one edge
    layout/metadata set serves all three layers.

Per-edge mechanics (both streams): edges grouped into 128-edge chunks
sharing a dst tile; chunks gathered 1024 indices per dma_gather from
32768-row buckets (int16 index range); VectorE builds one-hot
S^T[e,d] = (iota[d] == dst_local[e]) in bf16; TensorE accumulates
aggT[f,d] += msg^T @ S^T in PSUM.  Remainders (<128 edges per
(tile,bucket)) are merged across the whole window (GSG supergroups) to
cut padding; per-(tile, merged-chunk) overlaps become separate matmul
ops whose dst metadata (sentinel 999) masks foreign edges.
"""

import numpy as np

# ----------------------------------------------------------------- config

FULL_CFG = dict(
    N=100000,          # nodes
    D=128,             # feature dim (= hidden)
    CORES=8,
    TPG=4,             # dst tiles per supergroup (PSUM bank = 512 fp32)
    GSG=2,             # supergroups per window (remainder-merge scope)
    BUCKET=32768,      # source rows per gather bucket (int16 idx range)
    GIDX=1024,         # indices per dma_gather instruction
)


def _derive(cfg):
    c = dict(cfg)
    c["NPC"] = c["N"] // c["CORES"]            # nodes per core
    assert c["NPC"] * c["CORES"] == c["N"]
    c["NT"] = (c["NPC"] + 127) // 128          # dst tiles per core
    c["NSG"] = (c["NT"] + c["TPG"] - 1) // c["TPG"]
    c["NW"] = (c["NSG"] + c["GSG"] - 1) // c["GSG"]
    c["NB"] = (c["N"] + c["BUCKET"] - 1) // c["BUCKET"]
    return c


# ----------------------------------------------------- host preprocessing

def _stream_layout(cfg, NB, dst_core, tile, bucket):
    """SPMD slot/op layout for one edge stream.

    Slot order: window-major, then bucket; within a bucket, each tile's
    FULL 128-edge chunks first (tiles in sg order), then one merged
    region packing all window tiles' remainders.  One gather range per
    (window, bucket).
    """
    CORES, NT, TPG, GSG = cfg["CORES"], cfg["NT"], cfg["TPG"], cfg["GSG"]
    NSG = cfg["NSG"]
    key = (dst_core * NT * NB + tile * NB + bucket).astype(np.int64)
    counts = np.bincount(key, minlength=CORES * NT * NB)
    counts = counts.reshape(CORES, NT, NB)
    mx = counts.max(axis=0)                     # [NT, NB]
    full = mx // 128
    rem = mx % 128

    full_slot0 = np.zeros((NT, NB), np.int64)
    full_op0 = np.zeros((NT, NB), np.int64)
    full128 = (full * 128).astype(np.int64)
    rem_pos0 = np.zeros((NT, NB), np.int64)
    rem_slot0 = np.zeros((NT, NB), np.int64)
    rem_op0 = np.zeros((NT, NB), np.int64)

    wins = [list(range(w * GSG, min((w + 1) * GSG, NSG)))
            for w in range(cfg["NW"])]
    winfo = []
    goff = 0          # global slot offset
    ooff = 0          # global op-column offset
    for w, sgs_idx in enumerate(wins):
        gathers = []                      # (bucket, s0_local, n_chunks)
        sgs = [dict(sg=sg,
                    tiles=list(range(sg * TPG, min((sg + 1) * TPG, NT))),
                    tile_ops={}) for sg in sgs_idx]
        for s in sgs:
            s["tile_ops"] = {t: [] for t in s["tiles"]}
        off = 0
        opo = 0
        for b in range(NB):
            s0 = off
            for s in sgs:
                for t in s["tiles"]:
                    f = int(full[t, b])
                    full_slot0[t, b] = goff + off
                    full_op0[t, b] = ooff + opo
                    for j in range(f):
                        s["tile_ops"][t].append((off + j, opo + j))
                    off += f
                    opo += f
            # merged remainder region across the whole window
            cum = 0
            mslot0 = off
            for s in sgs:
                for t in s["tiles"]:
                    r = int(rem[t, b])
                    if r == 0:
                        continue
                    rem_pos0[t, b] = cum
                    rem_slot0[t, b] = goff + mslot0
                    rem_op0[t, b] = ooff + opo
                    first, last = cum // 128, (cum + r - 1) // 128
                    for m in range(first, last + 1):
                        s["tile_ops"][t].append((mslot0 + m, opo + (m - first)))
                    opo += last - first + 1
                    cum += r
            off += -(-cum // 128)
            if off > s0:
                gathers.append((b, s0, off - s0))
        winfo.append(dict(sgs=sgs, S_w=off, OPS_w=opo, goff=goff,
                          ops_off=ooff, gathers=gathers))
        goff += off
        ooff += opo
    return dict(winfo=winfo, tot_slots=goff, tot_ops=ooff, key=key,
                full_slot0=full_slot0, full_op0=full_op0, full128=full128,
                rem_pos0=rem_pos0, rem_slot0=rem_slot0, rem_op0=rem_op0)


def _stream_blobs(cfg, lay, NB, rows_inb, dst_core, tile, bucket, dst_inb):
    """Per-core idx/dst blobs for one stream.

    idx16: [CORES, 128, tot*8] int16 in the SWDGE wrapped layout: global
    gather position i (slot s = i//128, partition p = i%128) lives at
    [p%16 (+16k replicas), s*8 + p//16].
    dstm: [CORES, 128, tot_ops] fp32, 999.0 on pad partitions so the
    is_equal one-hot build masks them.
    """
    CORES = cfg["CORES"]
    tot = lay["tot_slots"]
    tot_ops = lay["tot_ops"]
    key = lay["key"]

    order = np.argsort(key, kind="stable")
    counts_flat = np.bincount(key, minlength=CORES * cfg["NT"] * NB)
    seg_off = np.concatenate([[0], np.cumsum(counts_flat)])
    rank_sorted = np.arange(len(order)) - seg_off[key[order]]
    rank = np.empty(len(order), np.int64)
    rank[order] = rank_sorted

    f128 = lay["full128"][tile, bucket]
    is_full = rank < f128
    pos = lay["rem_pos0"][tile, bucket] + (rank - f128)
    gslot = np.where(is_full,
                     lay["full_slot0"][tile, bucket] + rank // 128,
                     lay["rem_slot0"][tile, bucket] + pos // 128)
    part = np.where(is_full, rank % 128, pos % 128)
    opcol = np.where(is_full,
                     lay["full_op0"][tile, bucket] + rank // 128,
                     lay["rem_op0"][tile, bucket]
                     + pos // 128 - lay["rem_pos0"][tile, bucket] // 128)

    idx16 = np.zeros((CORES, 16, tot * 8), np.int16)
    dstm = np.full((CORES, 128, tot_ops), 999.0, np.float32)
    idx16[dst_core, part % 16, gslot * 8 + part // 16] = rows_inb.astype(
        np.int16)
    dstm[dst_core, part, opcol] = dst_inb.astype(np.float32)
    idx128 = np.tile(idx16, (1, 8, 1))          # replicate across Q7 cores
    return idx128, dstm


def preprocess(cfg, edge_index):
    N, CORES, NPC, NT = cfg["N"], cfg["CORES"], cfg["NPC"], cfg["NT"]
    BUCKET, NB = cfg["BUCKET"], cfg["NB"]
    ei = np.asarray(edge_index).astype(np.int64)
    src, dst = ei[0], ei[1]
    deg = (np.bincount(dst, minlength=N) + 1.0).astype(np.float64)
    dinv = 1.0 / np.sqrt(deg)

    # balanced node -> (core, local row) assignment: snake-deal nodes in
    # in-degree order across the CORES*NT (core, tile) bins so every
    # core's tile t sees a near-identical in-degree profile -- this is
    # what keeps the SPMD max-over-cores slot layout tight.
    order = np.argsort(-deg, kind="stable")
    nbins = CORES * NT
    binid = np.empty(N, np.int64)
    pos_in_bin = np.empty(N, np.int64)
    idx = np.arange(N)
    rnd = idx // nbins            # deal round
    col = idx % nbins
    snake = np.where(rnd % 2 == 0, col, nbins - 1 - col)
    binid[order] = snake
    pos_in_bin[order] = rnd
    own_core = binid % CORES
    tile_of = binid // CORES
    loc_v = tile_of * 128 + pos_in_bin
    keep = loc_v < NPC            # last tile is short (NPC % 128 != 0)
    # nodes dealt past NPC wrap into earlier free slots of their core
    for c in range(CORES):
        mc = own_core == c
        over = mc & ~keep
        if over.any():
            used = np.zeros(NPC, bool)
            used[loc_v[mc & keep]] = True
            free = np.flatnonzero(~used)
            loc_v[over] = free[:over.sum()]
    prow_v = own_core * NPC + loc_v

    oc = lambda v: own_core[v]
    loc = lambda v: loc_v[v]
    prow = prow_v[src]

    dst_core = oc(dst)
    dst_local = loc(dst)
    tile = dst_local // 128
    dst_inb = dst_local % 128

    own = oc(src) == dst_core
    # The main stream gathers from the (core-major) AllGather table, so a
    # bucket never contains the gathering core's own rows symmetrically:
    # cores whose slice lies outside bucket b systematically see more
    # bucket-b edges than in-bucket cores, and the SPMD max-over-cores
    # layout would pad the difference.  Use own-source edges as a
    # balancing valve: move just enough of them into the main stream to
    # lift every (core, tile, bucket) cell to the cross-edge max; only
    # the surplus stays in the AllGather-independent own stream.
    bsrc = prow // BUCKET
    cellk = (dst_core * NT + tile) * NB + bsrc
    ncell = CORES * NT * NB
    cnt_x = np.bincount(cellk[~own], minlength=ncell).reshape(
        CORES, NT, NB)
    target = cnt_x.max(axis=0)                  # [NT, NB]
    deficit = (target[None] - cnt_x).reshape(-1)
    eo = np.flatnonzero(own)
    ko = cellk[eo]
    o_order = np.argsort(ko, kind="stable")
    o_cnt = np.bincount(ko, minlength=ncell)
    o_off = np.concatenate([[0], np.cumsum(o_cnt)])
    o_rank = np.empty(len(eo), np.int64)
    o_rank[o_order] = np.arange(len(eo)) - o_off[ko[o_order]]
    moved = o_rank < deficit[ko]
    own[eo[moved]] = False

    m = ~own
    lay_m = _stream_layout(cfg, NB, dst_core[m], tile[m], prow[m] // BUCKET)
    blob_m = _stream_blobs(cfg, lay_m, NB, prow[m] % BUCKET, dst_core[m],
                           tile[m], prow[m] // BUCKET, dst_inb[m])
    # own stream: same-core sources, gathered from the local slice
    z = np.zeros(own.sum(), np.int64)
    lay_o = _stream_layout(cfg, 1, dst_core[own], tile[own], z)
    # PJRT rejects zero-size tensors: keep at least one (never-read)
    # slot/op column in the blobs when the stream is empty.
    lay_o["tot_slots"] = max(lay_o["tot_slots"], 1)
    lay_o["tot_ops"] = max(lay_o["tot_ops"], 1)
    blob_o = _stream_blobs(cfg, lay_o, 1, loc(src[own]), dst_core[own],
                           tile[own], z, dst_inb[own])
    return lay_m, lay_o, blob_m, blob_o, dinv, own_core, loc_v


# -------------------------------------------------------- device program

def build_program(cfg, lay_m, lay_o, n_layers=3, use_collective=True,
                  use_bias=True):
    import concourse.bass as bass  # noqa: F401
    import concourse.bacc as bacc
    import concourse.tile as tile
    import concourse.mybir as mybir

    f32 = mybir.dt.float32
    bf16 = mybir.dt.bfloat16
    i16 = mybir.dt.int16
    N, D, CORES = cfg["N"], cfg["D"], cfg["CORES"]
    NPC, NT, TPG = cfg["NPC"], cfg["NT"], cfg["TPG"]
    NB, BUCKET, GIDX = cfg["NB"], cfg["BUCKET"], cfg["GIDX"]
    GS = GIDX // 128                   # slots per gather instruction

    nc = bacc.Bacc("TRN2", target_bir_lowering=False, debug=False,
                   num_devices=CORES)

    x0p = nc.dram_tensor("x0p", [N, D], bf16, kind="ExternalInput")
    x0own = nc.dram_tensor("x0own", [NPC, D], bf16, kind="ExternalInput")
    x0T = nc.dram_tensor("x0T", [128, NT * 128], bf16, kind="ExternalInput")
    iota_in = nc.dram_tensor("iota", [128, 128], bf16, kind="ExternalInput")
    ident_in = nc.dram_tensor("ident", [128, 128], bf16,
                              kind="ExternalInput")
    W_in = [nc.dram_tensor(f"W{l}", [D, D], bf16, kind="ExternalInput")
            for l in range(3)]
    B_in = [nc.dram_tensor(f"b{l}", [1, D], bf16, kind="ExternalInput")
            for l in range(3)]
    invd_in = nc.dram_tensor("invd", [1, NT * 128], bf16,
                             kind="ExternalInput")
    dinvT_in = nc.dram_tensor("dinvT", [128, NT], f32, kind="ExternalInput")
    dinv2T_in = nc.dram_tensor("dinv2T", [128, NT], f32,
                               kind="ExternalInput")
    lays = {"m": lay_m, "o": lay_o}
    idx_in = {k: nc.dram_tensor(f"idx_{k}", [128, lays[k]["tot_slots"] * 8],
                                i16, kind="ExternalInput") for k in lays}
    dst_in = {k: nc.dram_tensor(f"dst_{k}", [128, lays[k]["tot_ops"]],
                                f32, kind="ExternalInput") for k in lays}
    y_out = nc.dram_tensor("y_out", [NPC, D], f32, kind="ExternalOutput")

    xs = [nc.dram_tensor(f"xslice{l}", [NPC, D], bf16) for l in range(2)]
    xg = [nc.dram_tensor(f"xgath{l}", [N, D], bf16, addr_space="Shared")
          for l in range(2)]

    with tile.TileContext(nc) as tc:
        with (
            tc.tile_pool(name="const", bufs=1) as constp,
            tc.tile_pool(name="msgm", bufs=3) as msgmp,
            tc.tile_pool(name="msgo", bufs=2) as msgop,
            tc.tile_pool(name="meta", bufs=3) as metap,
            tc.tile_pool(name="st", bufs=24) as stp,
            tc.tile_pool(name="sb2", bufs=12) as sb2p,
            tc.tile_pool(name="xsT", bufs=1) as xstp,
            tc.tile_pool(name="aggo", bufs=1) as aggop,
            tc.tile_pool(name="psA", bufs=3, space="PSUM") as psAp,
            tc.tile_pool(name="psY", bufs=3, space="PSUM") as psYp,
            tc.tile_pool(name="psT", bufs=2, space="PSUM") as psTp,
        ):
            iota_sb = constp.tile([128, 128], bf16, tag="iota")
            nc.sync.dma_start(iota_sb[:], iota_in[:, :])
            ident_sb = constp.tile([128, 128], bf16, tag="ident")
            nc.sync.dma_start(ident_sb[:], ident_in[:, :])
            dinvT = constp.tile([128, NT], f32, tag="dinvT")
            nc.sync.dma_start(dinvT[:], dinvT_in[:, :])
            dinv2T = constp.tile([128, NT], f32, tag="dinv2T")
            nc.sync.dma_start(dinv2T[:], dinv2T_in[:, :])
            W_sb, B_sb = [], []
            for l in range(3):
                w = constp.tile([128, 128], bf16, tag=f"W{l}")
                nc.sync.dma_start(w[:], W_in[l][:, :])
                W_sb.append(w)
                b = constp.tile([1, 128], bf16, tag=f"b{l}")
                nc.sync.dma_start(b[:], B_in[l][:, :])
                B_sb.append(b)

            # t_l^T for the core's own slice (self-loop seed + next layer)
            xsT = xstp.tile([128, NT * 128], bf16, tag="xsT")
            nc.sync.dma_start(xsT[:], x0T[:, :])

            def stream_ops(lay, key, x_src, bucket_rows, aggdst, psrc_merge):
                """Emit gathers + one-hot matmul ops for one stream/layer.

                aggdst(sg_dict, psA) is called once per supergroup after
                its accumulation completes.
                """
                for wi in lay["winfo"]:
                    if wi["S_w"] == 0:
                        continue
                    S_w = wi["S_w"]
                    OPS_w = wi["OPS_w"]
                    goff = wi["goff"]
                    ooff = wi["ops_off"]
                    pool = msgmp if key == "m" else msgop
                    msgb = pool.tile([128, S_w * 128], bf16, tag=f"msg{key}")
                    msgb3 = msgb[:].rearrange("p (s e) -> p s e", e=128)
                    idxt = metap.tile([128, S_w * 8], i16, tag=f"idx{key}")
                    dstt = metap.tile([128, OPS_w], f32, tag=f"dst{key}")
                    nc.sync.dma_start(idxt[:, :],
                                      idx_in[key][:, goff * 8:(goff + S_w) * 8])
                    nc.sync.dma_start(dstt[:, :],
                                      dst_in[key][:, ooff:ooff + OPS_w])
                    wctx = None
                    if key == "m":
                        t0w = wi["sgs"][0]["sg"] * TPG
                        t1w = wi["sgs"][-1]["tiles"][-1] + 1
                        invd_w = metap.tile([1, (t1w - t0w) * 128], bf16,
                                            tag="invd")
                        nc.sync.dma_start(
                            invd_w[:],
                            invd_in[:, t0w * 128:t1w * 128])
                        wctx = (invd_w, t0w)
                    for (b, g0, gch) in wi["gathers"]:
                        base = b * bucket_rows
                        rows = min(bucket_rows, x_src.shape[0] - base)
                        for s0 in range(g0, g0 + gch, GS):
                            nch = min(GS, g0 + gch - s0)
                            nidx = nch * 128
                            nc.gpsimd.dma_gather(
                                msgb3[:, s0:s0 + nch, :],
                                x_src[base:base + rows, :],
                                idxt[:, s0 * 8:(s0 + nch) * 8],
                                nidx, nidx, 128,
                            )
                    for sginfo in wi["sgs"]:
                        tiles = sginfo["tiles"]
                        ntl = len(tiles)
                        has_ops = any(sginfo["tile_ops"][t] for t in tiles)
                        if has_ops:
                            psA = psAp.tile([128, TPG * 128], f32, tag="psA")
                        else:
                            psA = None
                        for ti, t in enumerate(tiles):
                            ops = sginfo["tile_ops"][t]
                            for j, (s, oc_) in enumerate(ops):
                                stt = stp.tile([128, 128], bf16, tag="st")
                                nc.vector.tensor_scalar(
                                    stt[:], iota_sb[:],
                                    dstt[:, oc_:oc_ + 1], None,
                                    mybir.AluOpType.is_equal,
                                )
                                nc.tensor.matmul(
                                    psA[:, ti * 128:(ti + 1) * 128],
                                    msgb3[:, s, :], stt[:],
                                    start=(j == 0),
                                    stop=(j == len(ops) - 1),
                                )
                        aggdst(sginfo, psA, ntl, wctx)

            cnv = [0]

            def own_sink(sginfo, psA, ntl, wctx):
                # aggT_own[sg] = t_l^T[sg] + own-edge aggregate; tiles
                # with no ops in this stream never wrote their psA
                # columns, so they just copy the self-loop seed.
                sg = sginfo["sg"]
                for ti, t in enumerate(sginfo["tiles"]):
                    c = sg * TPG * 128 + ti * 128
                    if sginfo["tile_ops"][t]:
                        nc.vector.tensor_tensor(
                            aggT_own[:, c:c + 128],
                            psA[:, ti * 128:(ti + 1) * 128],
                            xsT[:, c:c + 128],
                            mybir.AluOpType.add)
                    else:
                        nc.vector.tensor_copy(
                            aggT_own[:, c:c + 128], xsT[:, c:c + 128])

            def main_sink(l, tgt, scaleT, build_T):
                def sink(sginfo, psA, ntl, wctx):
                    invd_w, t0w = wctx
                    sg = sginfo["sg"]
                    c0 = sg * TPG * 128
                    aggT = sb2p.tile([128, TPG * 128], bf16, tag="aggT")
                    if all(sginfo["tile_ops"][t] for t in sginfo["tiles"]):
                        nc.vector.tensor_tensor(
                            aggT[:, :ntl * 128], psA[:, :ntl * 128],
                            aggT_own[:, c0:c0 + ntl * 128],
                            mybir.AluOpType.add)
                    else:
                        for ti, t in enumerate(sginfo["tiles"]):
                            c = c0 + ti * 128
                            if sginfo["tile_ops"][t]:
                                nc.vector.tensor_tensor(
                                    aggT[:, ti * 128:(ti + 1) * 128],
                                    psA[:, ti * 128:(ti + 1) * 128],
                                    aggT_own[:, c:c + 128],
                                    mybir.AluOpType.add)
                            else:
                                nc.vector.tensor_copy(
                                    aggT[:, ti * 128:(ti + 1) * 128],
                                    aggT_own[:, c:c + 128])
                    for ti, t in enumerate(sginfo["tiles"]):
                        psY = psYp.tile([128, 128], f32, tag="psY")
                        if use_bias:
                            nc.tensor.matmul(psY[:],
                                             aggT[:, ti * 128:(ti + 1) * 128],
                                             W_sb[l][:], start=True,
                                             stop=False)
                            nc.tensor.matmul(
                                psY[:],
                                invd_w[0:1,
                                       (t - t0w) * 128:(t - t0w + 1) * 128],
                                B_sb[l][:], start=False, stop=True)
                        else:
                            nc.tensor.matmul(psY[:],
                                             aggT[:, ti * 128:(ti + 1) * 128],
                                             W_sb[l][:], start=True,
                                             stop=True)
                        ysb = sb2p.tile(
                            [128, 128], f32 if l == n_layers - 1 else bf16,
                            tag="ysb")
                        nc.scalar.mul(ysb[:], psY[:], scaleT[:, t:t + 1])
                        rows = min(128, NPC - t * 128)
                        nc.sync.dma_start(tgt[t * 128:t * 128 + rows, :],
                                          ysb[:rows, :])
                        if build_T:
                            psT = psTp.tile([128, 128], bf16, tag="psT")
                            nc.tensor.transpose(psT[:], ysb[:], ident_sb[:])
                            nc.scalar.copy(
                                xsT[:, t * 128:(t + 1) * 128], psT[:])
                return sink

            for l in range(n_layers):
                own_src = x0own if l == 0 else xs[l - 1]
                main_src = x0p if l == 0 else xg[l - 1]
                last = l == n_layers - 1
                tgt = y_out if last else xs[l]
                scaleT = dinvT if last else dinv2T
                aggT_own = aggop.tile([128, NT * 128], bf16, tag="aggo")
                # own stream first: no dependency on the previous AllGather
                stream_ops(lays["o"], "o", own_src, NPC, own_sink, None)
                stream_ops(lays["m"], "m", main_src, BUCKET,
                           main_sink(l, tgt, scaleT, not last), None)
                if not last and use_collective:
                    nc.gpsimd.collective_compute(
                        "AllGather",
                        mybir.AluOpType.bypass,
                        replica_groups=[list(range(CORES))],
                        ins=[xs[l][:, :].opt()],
                        outs=[xg[l][:, :].opt()],
                    )
    nc.compile()
    return nc


# ------------------------------------------------------------- execution

def make_in_maps(cfg, inputs, lay_m, lay_o, blob_m, blob_o, dinv,
                 own_core, loc_v):
    import ml_dtypes

    bf = ml_dtypes.bfloat16
    N, D, CORES, NPC, NT = (cfg["N"], cfg["D"], cfg["CORES"], cfg["NPC"],
                            cfg["NT"])
    idx_m, dst_m = blob_m
    idx_o, dst_o = blob_o
    iota = np.tile(np.arange(128, dtype=bf), (128, 1))
    ident = np.eye(128).astype(bf)

    x0 = np.asarray(inputs["node_features"], dtype=np.float64)
    x0s = x0 * dinv[:, None]
    # permuted table: row (v%8)*NPC + v//8 holds node v
    prow_v = own_core * NPC + loc_v
    x0p = np.empty((N, D), bf)
    x0p[prow_v] = x0s.astype(bf)
    dinvp = np.empty(N, np.float64)
    dinvp[prow_v] = dinv

    in_maps = []
    for c in range(CORES):
        sl = slice(c * NPC, (c + 1) * NPC)
        x0own = np.ascontiguousarray(x0p[sl])
        x0T = np.zeros((128, NT * 128), bf)
        x0T[:, :NPC] = x0own.T
        dv = dinvp[sl]
        dvT = np.zeros((128, NT), np.float32)
        dv2T = np.zeros((128, NT), np.float32)
        dpad = np.zeros(NT * 128)
        dpad[:NPC] = dv
        dvT[:, :] = dpad.reshape(NT, 128).T
        dv2T[:, :] = (dpad ** 2).reshape(NT, 128).T
        invd = np.zeros((1, NT * 128), bf)
        invd[0, :NPC] = (1.0 / dv).astype(bf)
        m = {
            "x0p": x0p,
            "x0own": x0own,
            "x0T": x0T,
            "iota": iota,
            "ident": ident,
            "invd": invd,
            "dinvT": dvT,
            "dinv2T": dv2T,
            "idx_m": np.ascontiguousarray(idx_m[c]),
            "dst_m": np.ascontiguousarray(dst_m[c]),
            "idx_o": np.ascontiguousarray(idx_o[c]),
            "dst_o": np.ascontiguousarray(dst_o[c]),
        }
        for l in range(3):
            m[f"W{l}"] = np.asarray(inputs[f"W{l}"], dtype=np.float32).astype(
                bf)
            m[f"b{l}"] = np.asarray(inputs[f"b{l}"],
                                    dtype=np.float32).reshape(1, D).astype(bf)
        in_maps.append(m)
    return in_maps


def unshard_output(cfg, results, own_core, loc_v):
    N, D = cfg["N"], cfg["D"]
    out = np.empty((N, D), np.float32)
    for c in range(cfg["CORES"]):
        m = own_core == c
        out[m] = results[c]["y_out"][loc_v[m]]
    return out


_CACHE = {}


def kernel(**inputs) -> np.ndarray:
    import time

    cfg = _derive(FULL_CFG)
    # the bias matmuls are dead code when every b_l is zero (the case
    # for this model's inputs); specialize the program accordingly
    use_bias = any(np.any(np.asarray(inputs[f"b{l}"])) for l in range(3))
    ekey = hash(np.asarray(inputs["edge_index"]).tobytes()) ^ use_bias
    if ekey in _CACHE:
        pre, nc = _CACHE[ekey]
    else:
        pre = preprocess(cfg, inputs["edge_index"])
        nc = build_program(cfg, pre[0], pre[1], use_bias=use_bias)
        _CACHE.clear()
        _CACHE[ekey] = (pre, nc)
    in_maps = make_in_maps(cfg, inputs, *pre)
    own_core, loc_v = pre[5], pre[6]
    from concourse import bass_utils

    # The axon-tunneled device occasionally dies mid-run
    # (NRT_EXEC_UNIT_UNRECOVERABLE) and the worker restarts itself over
    # the next minute or two; retry a few times before giving up.
    last_exc = None
    for attempt, backoff_s in enumerate([0, 90, 180, 240]):
        if backoff_s:
            time.sleep(backoff_s)
        try:
            res = bass_utils.run_bass_kernel_spmd(
                nc, in_maps, core_ids=list(range(cfg["CORES"])))
            return unshard_output(cfg, res.results, own_core, loc_v)
        except Exception as exc:  # transient worker/device failures
            last_exc = exc
            try:
                import jax
                jax.clear_caches()
            except Exception:
                pass
    raise last_exc
